# revision 1
# baseline (speedup 1.0000x reference)
"""Trainium2 Bass kernel for nn_AutoShot (histogram binning + windowed similarity + FC).

Sharding: data-parallel over B*T = 400 frames -> 8 cores x 50 frames.
Phase A (heavy): per-core color histograms [50, 512] via
  bin = (R>>5)<<6 | (G>>5)<<3 | (B>>5), split bin = hi5*16 + lo4,
  one-hot(hi5) [px,32] x one-hot(lo4) [px,16] contracted over pixels on the
  PE (PSUM-accumulated bf16 matmuls) -> joint 2-D histogram [32,16] = hist[512].
Phase B (light): per-core sim = xh @ xs^T (xs = zero-padded +-50 frame context),
  diagonal window extract via a stride-164 read over stride-163 rows in a DRAM
  scratch (addr 164*t + l = sim[t, t+l]), PE transpose, FC matmul (W [128,101]).
Host: slices inputs, L2-normalizes histograms between launches, applies
  bias + ReLU (tiny [400,128] tail), reassembles the [4,100,128] output.
"""

import sys

for _p in ("/opt/trn_rl_repo", "/root/.axon_site/_ro/trn_rl_repo"):
    if _p not in sys.path:
        sys.path.append(_p)

import numpy as np

from concourse import bass, bacc, mybir
import concourse.tile as tile
from concourse.bass_utils import run_bass_kernel_spmd
from concourse.masks import make_identity

P = 128
NPIX = 224 * 224        # 50176 pixels per frame plane
FPP = NPIX // P         # 392 pixels per partition
NF = 50                 # frames per core
V1, V2 = 32, 16         # 512 = 32 * 16 bin split
LW = 101
NCORES = 8
F32 = mybir.dt.float32
I32 = mybir.dt.int32
BF16 = mybir.dt.bfloat16
OP = mybir.AluOpType


def build_hist_nc():
    nc = bacc.Bacc("TRN2")
    fr = nc.dram_tensor("fr", [3, NF, NPIX], I32, kind="ExternalInput")
    hist = nc.dram_tensor("hist", [NF, 512], F32, kind="ExternalOutput")
    G = 2                # frames per DVE batch (amortizes per-op overhead)
    FD = G * FPP         # 784 free-dim elements per DVE op

    with tile.TileContext(nc) as tc:
        with (
            tc.tile_pool(name="io", bufs=4) as io,
            tc.tile_pool(name="mid", bufs=2) as mid,
            tc.tile_pool(name="oh", bufs=2) as oh,
            tc.tile_pool(name="cst", bufs=1) as cst,
            tc.tile_pool(name="ps", bufs=2, space="PSUM") as ps,
        ):
            osb = cst.tile([V1, NF * V2], F32)  # [32, 800] result staging

            for t0 in range(0, NF, G):
                r = io.tile([P, FD], I32, tag="ch")
                g = io.tile([P, FD], I32, tag="ch")
                b = io.tile([P, FD], I32, tag="ch")
                for ci, ch in ((0, r), (1, g), (2, b)):
                    nc.sync.dma_start(
                        out=ch[:].rearrange("p (q f) -> p q f", q=G),
                        in_=fr[ci, t0:t0 + G].rearrange("q (p f) -> p q f", p=P))

                # hi5 = (R>>5)*4 + (G>>6) = ((R>>3)&28) | (G>>6)
                # lo4 = ((G>>5)&1)*8 + (B>>5) = ((G>>2)&8) | (B>>5)
                a2 = mid.tile([P, FD], I32, tag="t1")
                nc.vector.tensor_scalar(
                    out=a2[:], in0=r[:], scalar1=3, scalar2=28,
                    op0=OP.logical_shift_right, op1=OP.bitwise_and)
                b2 = mid.tile([P, FD], I32, tag="t2")
                nc.vector.tensor_scalar(
                    out=b2[:], in0=g[:], scalar1=6, scalar2=None,
                    op0=OP.logical_shift_right)
                hi_i = mid.tile([P, FD], I32, tag="t3")
                nc.vector.tensor_tensor(
                    out=hi_i[:], in0=a2[:], in1=b2[:], op=OP.bitwise_or)
                hi_b = mid.tile([P, FD], BF16, tag="tb")
                nc.vector.tensor_copy(out=hi_b[:], in_=hi_i[:])

                c2 = mid.tile([P, FD], I32, tag="t1")
                nc.vector.tensor_scalar(
                    out=c2[:], in0=g[:], scalar1=2, scalar2=8,
                    op0=OP.logical_shift_right, op1=OP.bitwise_and)
                d2 = mid.tile([P, FD], I32, tag="t2")
                nc.vector.tensor_scalar(
                    out=d2[:], in0=b[:], scalar1=5, scalar2=None,
                    op0=OP.logical_shift_right)
                lo_i = mid.tile([P, FD], I32, tag="t3")
                nc.vector.tensor_tensor(
                    out=lo_i[:], in0=c2[:], in1=d2[:], op=OP.bitwise_or)
                lo_b = mid.tile([P, FD], BF16, tag="tb")
                nc.vector.tensor_copy(out=lo_b[:], in_=lo_i[:])

                # one-hot via per-value tensor_scalar is_equal over G frames:
                # bf16 single-src step-1 SBUF -> DVE 4x perf mode.
                A = oh.tile([P, V1 * FD], BF16, tag="A")
                for v in range(V1):
                    nc.vector.tensor_scalar(
                        out=A[:, v * FD:(v + 1) * FD], in0=hi_b[:],
                        scalar1=float(v), scalar2=None, op0=OP.is_equal)
                B = oh.tile([P, V2 * FD], BF16, tag="B")
                for v in range(V2):
                    nc.vector.tensor_scalar(
                        out=B[:, v * FD:(v + 1) * FD], in0=lo_b[:],
                        scalar1=float(v), scalar2=None, op0=OP.is_equal)

                # contract over pixels per frame: hist2d[u, w] += A_qj^T @ B_qj
                Aq = A[:].rearrange("p (v q f) -> p q f v", v=V1, q=G)
                Bq = B[:].rearrange("p (v q f) -> p q f v", v=V2, q=G)
                for q in range(G):
                    hps = ps.tile([V1, V2], F32)
                    for j in range(FPP):
                        nc.tensor.matmul(
                            out=hps[:],
                            lhsT=Aq[:, q, j, :],
                            rhs=Bq[:, q, j, :],
                            start=(j == 0), stop=(j == FPP - 1))
                    t = t0 + q
                    nc.vector.tensor_copy(
                        out=osb[:, t * V2:(t + 1) * V2], in_=hps[:])

            nc.sync.dma_start(
                out=hist[:].rearrange("t (u w) -> u t w", u=V1),
                in_=osb[:].rearrange("u (t w) -> u t w", w=V2))
    nc.compile()
    return nc


def build_fc_nc():
    """sim2 = xh @ xs^T [50,150]; win[t,l] = sim2[t, t+l]; out = relu(win@W^T + b)."""
    nc = bacc.Bacc("TRN2")
    # columns 0:50 = x_half^T, 50:200 = padded-context^T (one DMA -> one sem wait)
    xallT = nc.dram_tensor("xallT", [512, 200], F32, kind="ExternalInput")
    wT = nc.dram_tensor("wT", [LW, P], F32, kind="ExternalInput")
    out = nc.dram_tensor("out", [NF, P], F32, kind="ExternalOutput")
    # rows written at stride 163 (sim2[t] at 163*t), diagonal read back at
    # stride 164: addr 164*t + l = 163*t + (t+l) = sim2[t, t+l]  (no overlap)
    scratch = nc.dram_tensor("scratch", [NF * 164], F32, kind="Internal")

    with tile.TileContext(nc) as tc:
        with (
            tc.tile_pool(name="sb", bufs=1) as sb,
            tc.tile_pool(name="ps", bufs=1, space="PSUM") as ps,
        ):
            xa_sb = sb.tile([P, 4 * 200], F32)
            nc.sync.dma_start(
                out=xa_sb[:].rearrange("p (a t) -> p a t", a=4),
                in_=xallT[:].rearrange("(a p) t -> p a t", p=P))
            wt_sb = sb.tile([LW, P], F32)
            nc.sync.dma_start(out=wt_sb[:], in_=wT[:])

            sim_ps = ps.tile([NF, 150], F32)
            for a in range(4):
                nc.tensor.matmul(
                    out=sim_ps[:],
                    lhsT=xa_sb[:, a * 200:a * 200 + NF],
                    rhs=xa_sb[:, a * 200 + NF:(a + 1) * 200],
                    start=(a == 0), stop=(a == 3))
            sim_sb = sb.tile([NF, 150], F32)
            nc.vector.tensor_copy(out=sim_sb[:], in_=sim_ps[:])

            # row t of sim2 lands at flat offset 163*t
            nc.gpsimd.dma_start(
                out=scratch[0:NF * 163].rearrange("(t c) -> t c", c=163)[:, 0:150],
                in_=sim_sb[:])
            # diagonal: win[t, l] = scratch[164*t + l] = sim2[t, t+l]
            win_sb = sb.tile([NF, LW], F32)
            nc.gpsimd.dma_start(
                out=win_sb[:],
                in_=scratch[0:NF * 164].rearrange("(t c) -> t c", c=164)[:, 0:LW])

            # transpose win [50, 101] -> [101, 50] on the PE
            ident = sb.tile([NF, NF], F32)
            make_identity(nc, ident[:])
            win_ps = ps.tile([LW, NF], F32)
            nc.tensor.transpose(out=win_ps[:], in_=win_sb[:], identity=ident[:])
            win2 = sb.tile([LW, NF], F32)
            nc.vector.tensor_copy(out=win2[:], in_=win_ps[:])
            wt2 = sb.tile([LW, P], F32)
            nc.vector.tensor_copy(out=wt2[:], in_=wt_sb[:])

            fc_ps = ps.tile([P, NF], F32)
            nc.tensor.matmul(out=fc_ps[:], lhsT=wt2[:], rhs=win2[:],
                             start=True, stop=True)
            res = sb.tile([P, NF], F32)
            nc.vector.tensor_copy(out=res[:], in_=fc_ps[:])
            # bias + relu applied on host (tiny); avoids a 2-wait Activation
            nc.sync.dma_start(out=out[:].rearrange("t o -> o t"), in_=res[:])
    nc.compile()
    return nc


_NC_CACHE = {}


def _get_nc(key, builder):
    if key not in _NC_CACHE:
        _NC_CACHE[key] = builder()
    return _NC_CACHE[key]


def kernel(frames, W, b):
    frames = np.asarray(frames, dtype=np.int32)
    W = np.asarray(W, dtype=np.float32)
    b = np.asarray(b, dtype=np.float32)
    Bn, _, T = frames.shape[:3]  # [4, 3, 100, 224, 224]

    nc_a = _get_nc("A", build_hist_nc)
    in_maps = []
    for c in range(NCORES):
        bi, h = c // 2, c % 2
        sl = frames[bi, :, h * NF:(h + 1) * NF].reshape(3, NF, NPIX)
        in_maps.append({"fr": np.ascontiguousarray(sl)})
    res_a = run_bass_kernel_spmd(nc_a, in_maps, list(range(NCORES))).results

    counts = np.zeros((Bn, T, 512), np.float32)
    for c in range(NCORES):
        bi, h = c // 2, c % 2
        counts[bi, h * NF:(h + 1) * NF] = res_a[c]["hist"]
    xn = counts / np.linalg.norm(counts, axis=2, keepdims=True)

    nc_b = _get_nc("B", build_fc_nc)
    wT = np.ascontiguousarray(W.T)           # [101, 128]
    in_maps = []
    for c in range(NCORES):
        bi, h = c // 2, c % 2
        t0 = h * NF
        xall = np.zeros((200, 512), np.float32)
        xall[0:NF] = xn[bi, t0:t0 + NF]                  # x_half
        xall[NF + 50 - t0:NF + 50 - t0 + T] = xn[bi]     # xs[s'] = xn[s'+t0-50]
        in_maps.append({"xallT": np.ascontiguousarray(xall.T), "wT": wT})
    res_b = run_bass_kernel_spmd(nc_b, in_maps, list(range(NCORES))).results

    outp = np.zeros((Bn, T, P), np.float32)
    for c in range(NCORES):
        bi, h = c // 2, c % 2
        outp[bi, h * NF:(h + 1) * NF] = res_b[c]["out"]
    outp = np.maximum(outp + b[None, None, :], 0.0)
    return outp



# revision 18
# speedup vs baseline: 1.5261x; 1.5261x over previous
"""Trainium2 Bass kernel for nn_AutoShot (histogram binning + windowed similarity + FC).

Sharding: data-parallel over B*T = 400 frames -> 8 cores x 50 frames.
Phase A (heavy): per-core color histograms [50, 512] via
  bin = (R>>5)<<6 | (G>>5)<<3 | (B>>5), split bin = hi5*16 + lo4,
  encoding matrices A [px,32], B [px,16] contracted over pixels on the
  PE (PSUM-accumulated bf16 matmuls) -> joint 2-D histogram [32,16].
  Encoding columns are split across three engines to run concurrently:
    - DVE:  is_equal one-hot columns (4x perf mode, bf16)
    - Act:  relu-ramp columns relu((x-s)*c) - exact linear correction on host
    - Pool: is_equal one-hot columns
  The per-frame [32,16] raw moment matrix M = A^T B is corrected on the host:
  H = inv(VA^T) @ M @ inv(VB) where VA/VB are the exact encoding matrices
  (integer/eighth-integer entries, exact in bf16/fp32; correction in f64).
Phase B (light): per-core sim = xh @ xs^T (xs = zero-padded +-50 frame context),
  diagonal window extract via a stride-164 read over stride-163 rows in a DRAM
  scratch (addr 164*t + l = sim[t, t+l]), PE transpose, FC matmul (W [128,101]).
Host: slices inputs, applies correction, L2-normalizes histograms between
  launches, applies bias + ReLU (tiny [400,128] tail), reassembles output.
"""

import sys

for _p in ("/opt/trn_rl_repo", "/root/.axon_site/_ro/trn_rl_repo"):
    if _p not in sys.path:
        sys.path.append(_p)

import numpy as np

from concourse import bass, bacc, mybir
import concourse.tile as tile
from concourse.bass_utils import run_bass_kernel_spmd
from concourse.masks import make_identity

P = 128
NPIX = 224 * 224        # 50176 pixels per frame plane
FPP = NPIX // P         # 392 pixels per partition
NF = 50                 # frames per core
V1, V2 = 32, 16         # 512 = 32 * 16 bin split
LW = 101
NCORES = 8
F32 = mybir.dt.float32
I32 = mybir.dt.int32
I16 = mybir.dt.int16
BF16 = mybir.dt.bfloat16
OP = mybir.AluOpType
ACT = mybir.ActivationFunctionType

# Column assignment across engines (balanced by cost model rates):
#  DVE is_equal col: 265ns/batch; Act ramp col: 838ns; Pool is_equal col: 1090ns
A_DVE = list(range(28))       # A-side one-hot columns on DVE
A_POOL = list(range(28, 32))  # A-side one-hot columns on Pool (imm scalar)
A_ACT = []                    # (unused) A-side ramp cols
B_ACT = list(range(11))       # B-side ramp cols: relu((l-(w-1))/4), w=0..10
B_POOL = list(range(11, 16))  # B-side one-hot columns on Pool (imm scalar)


def encoding_mats():
    """Exact encoding matrices VA [32,32], VB [16,16]: col c of VA evaluated
    at value h is the device-computed encoding A[pix,c] for hi==h."""
    h = np.arange(32, dtype=np.float64)
    VA = np.zeros((32, 32))
    for c in A_DVE:
        VA[c, c] = 1.0
    for c in A_POOL:
        VA[c, c] = 1.0
    for c in A_ACT:
        VA[:, c] = np.maximum((h - (c - 1)) * 0.125, 0.0)
    ll = np.arange(16, dtype=np.float64)
    VB = np.zeros((16, 16))
    for c in B_ACT:
        VB[:, c] = np.maximum((ll - (c - 1)) * 0.25, 0.0)
    for c in B_POOL:
        VB[c, c] = 1.0
    return VA, VB


def _stt_int(nc, out, in0, scalar_int, in1, op0, op1):
    """scalar_tensor_tensor with an int32 immediate: out = (in0 op0 s) op1 in1."""
    v = nc.vector
    return v.add_instruction(mybir.InstTensorScalarPtr(
        name=v.bass.get_next_instruction_name(),
        is_scalar_tensor_tensor=True,
        op0=op0, op1=op1,
        ins=[v.lower_ap(in0),
             mybir.ImmediateValue(dtype=mybir.dt.int32, value=scalar_int),
             v.lower_ap(in1)],
        outs=[v.lower_ap(out)],
    ))


def build_hist_nc():
    nc = bacc.Bacc("TRN2")
    fr = nc.dram_tensor("fr", [3, NF, NPIX], I32, kind="ExternalInput")
    hist = nc.dram_tensor("hist", [NF, 512], F32, kind="ExternalOutput")
    G = 2                # frames per op batch (amortizes per-op overhead)
    FD = G * FPP         # 784 free-dim elements per op

    with tile.TileContext(nc) as tc:
        with (
            tc.tile_pool(name="io", bufs=4) as io,
            tc.tile_pool(name="mid", bufs=2) as mid,
            tc.tile_pool(name="oh", bufs=2) as oh,
            tc.tile_pool(name="cst", bufs=1) as cst,
            tc.tile_pool(name="ps", bufs=4, space="PSUM") as ps,
        ):
            osb = cst.tile([V1, NF * V2], F32)  # [32, 800] result staging

            # per-ramp-column bias constants for the Act engine ([128,1] each)
            nbias = len(A_ACT) + len(B_ACT)
            bias_sb = cst.tile([P, max(nbias, 1)], F32)
            bias_ap = {}
            bi_i = 0
            for v in A_ACT:
                nc.gpsimd.memset(bias_sb[:, bi_i:bi_i + 1], -(v - 1) * 0.125)
                bias_ap[("A", v)] = bias_sb[:, bi_i:bi_i + 1]
                bi_i += 1
            for w in B_ACT:
                nc.gpsimd.memset(bias_sb[:, bi_i:bi_i + 1], -(w - 1) * 0.25)
                bias_ap[("B", w)] = bias_sb[:, bi_i:bi_i + 1]
                bi_i += 1


            for t0 in range(0, NF, G):
                r = io.tile([P, FD], I32, tag="ch")
                g = io.tile([P, FD], I32, tag="ch")
                b = io.tile([P, FD], I32, tag="ch")
                for ci, ch in ((0, r), (1, g), (2, b)):
                    nc.sync.dma_start(
                        out=ch[:].rearrange("p (q f) -> p q f", q=G),
                        in_=fr[ci, t0:t0 + G].rearrange("q (p f) -> p q f", p=P))

                # hi5 = ((R>>3)&28) | (G>>6) ; lo4 = ((G>>2)&8) | (B>>5)
                a2 = mid.tile([P, FD], I32, tag="t1")
                nc.vector.tensor_scalar(
                    out=a2[:], in0=r[:], scalar1=3, scalar2=28,
                    op0=OP.logical_shift_right, op1=OP.bitwise_and)
                hi_i = mid.tile([P, FD], I32, tag="hb")
                _stt_int(nc, hi_i[:], g[:], 6, a2[:],
                         OP.logical_shift_right, OP.bitwise_or)
                hi = mid.tile([P, FD], BF16, tag="hc")
                nc.scalar.copy(out=hi[:], in_=hi_i[:])
                c2 = mid.tile([P, FD], I32, tag="t2")
                nc.vector.tensor_scalar(
                    out=c2[:], in0=g[:], scalar1=2, scalar2=8,
                    op0=OP.logical_shift_right, op1=OP.bitwise_and)
                lo_i = mid.tile([P, FD], I32, tag="lb")
                _stt_int(nc, lo_i[:], b[:], 5, c2[:],
                         OP.logical_shift_right, OP.bitwise_or)

                A = oh.tile([P, V1 * FD], BF16, tag="A")
                for v in A_DVE:
                    nc.vector.tensor_scalar(
                        out=A[:, v * FD:(v + 1) * FD], in0=hi[:],
                        scalar1=float(v), scalar2=None, op0=OP.is_equal)
                for v in A_POOL:
                    nc.gpsimd.tensor_scalar(
                        out=A[:, v * FD:(v + 1) * FD], in0=hi_i[:],
                        scalar1=float(v), scalar2=None, op0=OP.is_equal)
                B = oh.tile([P, V2 * FD], BF16, tag="B")
                for w in B_ACT:
                    nc.scalar.activation(
                        out=B[:, w * FD:(w + 1) * FD], in_=lo_i[:],
                        func=ACT.Relu, bias=bias_ap[("B", w)], scale=0.25)
                for w in B_POOL:
                    nc.gpsimd.tensor_scalar(
                        out=B[:, w * FD:(w + 1) * FD], in0=lo_i[:],
                        scalar1=float(w), scalar2=None, op0=OP.is_equal)

                # contract over pixels per frame: M[u, w] += A_qj^T @ B_qj
                Aq = A[:].rearrange("p (v q f) -> p q f v", v=V1, q=G)
                Bq = B[:].rearrange("p (v q f) -> p q f v", v=V2, q=G)
                for q in range(G):
                    hps = ps.tile([V1, V2], F32)
                    for j in range(FPP):
                        nc.tensor.matmul(
                            out=hps[:],
                            lhsT=Aq[:, q, j, :],
                            rhs=Bq[:, q, j, :],
                            start=(j == 0), stop=(j == FPP - 1))
                    t = t0 + q
                    nc.vector.tensor_copy(
                        out=osb[:, t * V2:(t + 1) * V2], in_=hps[:])

            nc.sync.dma_start(
                out=hist[:].rearrange("t (u w) -> u t w", u=V1),
                in_=osb[:].rearrange("u (t w) -> u t w", w=V2))
    nc.compile()
    return nc


def build_fc_nc():
    """sim2 = xh @ xs^T [50,150]; win[t,l] = sim2[t, t+l]; out = relu(win@W^T + b)."""
    nc = bacc.Bacc("TRN2")
    # columns 0:50 = x_half^T, 50:200 = padded-context^T (one DMA -> one sem wait)
    xallT = nc.dram_tensor("xallT", [512, 200], F32, kind="ExternalInput")
    wT = nc.dram_tensor("wT", [LW, P], F32, kind="ExternalInput")
    out = nc.dram_tensor("out", [NF, P], F32, kind="ExternalOutput")
    # rows written at stride 163 (sim2[t] at 163*t), diagonal read back at
    # stride 164: addr 164*t + l = 163*t + (t+l) = sim2[t, t+l]  (no overlap)
    scratch = nc.dram_tensor("scratch", [NF * 164], F32, kind="Internal")

    with tile.TileContext(nc) as tc:
        with (
            tc.tile_pool(name="sb", bufs=1) as sb,
            tc.tile_pool(name="ps", bufs=1, space="PSUM") as ps,
        ):
            xa_sb = sb.tile([P, 4 * 200], F32)
            nc.sync.dma_start(
                out=xa_sb[:].rearrange("p (a t) -> p a t", a=4),
                in_=xallT[:].rearrange("(a p) t -> p a t", p=P))
            wt_sb = sb.tile([LW, P], F32)
            nc.sync.dma_start(out=wt_sb[:], in_=wT[:])

            sim_ps = ps.tile([NF, 150], F32)
            for a in range(4):
                nc.tensor.matmul(
                    out=sim_ps[:],
                    lhsT=xa_sb[:, a * 200:a * 200 + NF],
                    rhs=xa_sb[:, a * 200 + NF:(a + 1) * 200],
                    start=(a == 0), stop=(a == 3))
            sim_sb = sb.tile([NF, 150], F32)
            nc.vector.tensor_copy(out=sim_sb[:], in_=sim_ps[:])

            # row t of sim2 lands at flat offset 163*t
            nc.gpsimd.dma_start(
                out=scratch[0:NF * 163].rearrange("(t c) -> t c", c=163)[:, 0:150],
                in_=sim_sb[:])
            # diagonal: win[t, l] = scratch[164*t + l] = sim2[t, t+l]
            win_sb = sb.tile([NF, LW], F32)
            nc.gpsimd.dma_start(
                out=win_sb[:],
                in_=scratch[0:NF * 164].rearrange("(t c) -> t c", c=164)[:, 0:LW])

            # transpose win [50, 101] -> [101, 50] on the PE
            ident = sb.tile([NF, NF], F32)
            make_identity(nc, ident[:])
            win_ps = ps.tile([LW, NF], F32)
            nc.tensor.transpose(out=win_ps[:], in_=win_sb[:], identity=ident[:])
            win2 = sb.tile([LW, NF], F32)
            nc.vector.tensor_copy(out=win2[:], in_=win_ps[:])
            wt2 = sb.tile([LW, P], F32)
            nc.vector.tensor_copy(out=wt2[:], in_=wt_sb[:])

            fc_ps = ps.tile([P, NF], F32)
            nc.tensor.matmul(out=fc_ps[:], lhsT=wt2[:], rhs=win2[:],
                             start=True, stop=True)
            res = sb.tile([P, NF], F32)
            nc.vector.tensor_copy(out=res[:], in_=fc_ps[:])
            # bias + relu applied on host (tiny); avoids a 2-wait Activation
            nc.sync.dma_start(out=out[:].rearrange("t o -> o t"), in_=res[:])
    nc.compile()
    return nc


_NC_CACHE = {}


def _get_nc(key, builder):
    if key not in _NC_CACHE:
        _NC_CACHE[key] = builder()
    return _NC_CACHE[key]


def kernel(frames, W, b):
    frames = np.asarray(frames, dtype=np.int32)
    W = np.asarray(W, dtype=np.float32)
    b = np.asarray(b, dtype=np.float32)
    Bn, _, T = frames.shape[:3]  # [4, 3, 100, 224, 224]

    nc_a = _get_nc("A", build_hist_nc)
    in_maps = []
    for c in range(NCORES):
        bi, h = c // 2, c % 2
        sl = frames[bi, :, h * NF:(h + 1) * NF].reshape(3, NF, NPIX)
        in_maps.append({"fr": np.ascontiguousarray(sl)})
    res_a = run_bass_kernel_spmd(nc_a, in_maps, list(range(NCORES))).results

    # exact correction of ramp-encoded columns: H = inv(VA^T) @ M @ inv(VB)
    VA, VB = encoding_mats()
    CA = np.linalg.inv(VA.T)
    CB = np.linalg.inv(VB)
    counts = np.zeros((Bn, T, 512), np.float32)
    for c in range(NCORES):
        bi, h = c // 2, c % 2
        M = res_a[c]["hist"].astype(np.float64).reshape(NF, V1, V2)
        H = np.einsum('uh,thl,lw->tuw', CA, M, CB)
        counts[bi, h * NF:(h + 1) * NF] = H.reshape(NF, 512)
    xn = counts / np.linalg.norm(counts, axis=2, keepdims=True)

    nc_b = _get_nc("B", build_fc_nc)
    wT = np.ascontiguousarray(W.T)           # [101, 128]
    in_maps = []
    for c in range(NCORES):
        bi, h = c // 2, c % 2
        t0 = h * NF
        xall = np.zeros((200, 512), np.float32)
        xall[0:NF] = xn[bi, t0:t0 + NF]                  # x_half
        xall[NF + 50 - t0:NF + 50 - t0 + T] = xn[bi]     # xs[s'] = xn[s'+t0-50]
        in_maps.append({"xallT": np.ascontiguousarray(xall.T), "wT": wT})
    res_b = run_bass_kernel_spmd(nc_b, in_maps, list(range(NCORES))).results

    outp = np.zeros((Bn, T, P), np.float32)
    for c in range(NCORES):
        bi, h = c // 2, c % 2
        outp[bi, h * NF:(h + 1) * NF] = res_b[c]["out"]
    outp = np.maximum(outp + b[None, None, :], 0.0)
    return outp


# revision 24
# speedup vs baseline: 1.5366x; 1.0069x over previous
"""Trainium2 Bass kernel for nn_AutoShot (histogram binning + windowed similarity + FC).

Sharding: data-parallel over B*T = 400 frames -> 8 cores x 50 frames.
Phase A (heavy): per-core color histograms [50, 512] via
  bin = (R>>5)<<6 | (G>>5)<<3 | (B>>5), split bin = hi5*16 + lo4,
  encoding matrices A [px,32], B [px,16] contracted over pixels on the
  PE (PSUM-accumulated bf16 matmuls) -> joint 2-D histogram [32,16].
  Encoding columns are split across three engines to run concurrently:
    - DVE:  is_equal one-hot columns (4x perf mode, bf16)
    - Act:  relu-ramp columns relu((x-s)*c) - exact linear correction on host
    - Pool: is_equal one-hot columns
  The per-frame [32,16] raw moment matrix M = A^T B is corrected on the host:
  H = inv(VA^T) @ M @ inv(VB) where VA/VB are the exact encoding matrices
  (integer/eighth-integer entries, exact in bf16/fp32; correction in f64).
Phase B (light): per-core sim = xh @ xs^T (xs = zero-padded +-50 frame context),
  diagonal window extract via a stride-164 read over stride-163 rows in a DRAM
  scratch (addr 164*t + l = sim[t, t+l]), PE transpose, FC matmul (W [128,101]).
Host: slices inputs, applies correction, L2-normalizes histograms between
  launches, applies bias + ReLU (tiny [400,128] tail), reassembles output.
"""

import sys

for _p in ("/opt/trn_rl_repo", "/root/.axon_site/_ro/trn_rl_repo"):
    if _p not in sys.path:
        sys.path.append(_p)

import numpy as np

from concourse import bass, bacc, mybir
import concourse.tile as tile
from concourse.bass_utils import run_bass_kernel_spmd
from concourse.masks import make_identity

P = 128
NPIX = 224 * 224        # 50176 pixels per frame plane
FPP = NPIX // P         # 392 pixels per partition
NF = 50                 # frames per core
V1, V2 = 32, 16         # 512 = 32 * 16 bin split
LW = 101
NCORES = 8
F32 = mybir.dt.float32
I32 = mybir.dt.int32
I16 = mybir.dt.int16
BF16 = mybir.dt.bfloat16
OP = mybir.AluOpType
ACT = mybir.ActivationFunctionType

# Column assignment across engines (balanced by cost model rates):
#  DVE is_equal col: 265ns/batch; Act ramp col: 838ns; Pool is_equal col: 1090ns
A_DVE = list(range(28))       # A-side one-hot columns on DVE
A_POOL = list(range(28, 32))  # A-side one-hot columns on Pool (imm scalar)
A_ACT = []                    # (unused) A-side ramp cols
B_ACT = list(range(11))       # B-side ramp cols: relu((l-(w-1))/4), w=0..10
B_POOL = list(range(11, 16))  # B-side one-hot columns on Pool (imm scalar)


def encoding_mats():
    """Exact encoding matrices VA [32,32], VB [16,16]: col c of VA evaluated
    at value h is the device-computed encoding A[pix,c] for hi==h."""
    h = np.arange(32, dtype=np.float64)
    VA = np.zeros((32, 32))
    for c in A_DVE:
        VA[c, c] = 1.0
    for c in A_POOL:
        VA[c, c] = 1.0
    for c in A_ACT:
        VA[:, c] = np.maximum((h - (c - 1)) * 0.125, 0.0)
    ll = np.arange(16, dtype=np.float64)
    VB = np.zeros((16, 16))
    for c in B_ACT:
        VB[:, c] = np.maximum((ll - (c - 1)) * 0.25, 0.0)
    for c in B_POOL:
        VB[c, c] = 1.0
    return VA, VB


def _stt_int(nc, out, in0, scalar_int, in1, op0, op1):
    """scalar_tensor_tensor with an int32 immediate: out = (in0 op0 s) op1 in1."""
    v = nc.vector
    return v.add_instruction(mybir.InstTensorScalarPtr(
        name=v.bass.get_next_instruction_name(),
        is_scalar_tensor_tensor=True,
        op0=op0, op1=op1,
        ins=[v.lower_ap(in0),
             mybir.ImmediateValue(dtype=mybir.dt.int32, value=scalar_int),
             v.lower_ap(in1)],
        outs=[v.lower_ap(out)],
    ))


def build_hist_nc():
    nc = bacc.Bacc("TRN2")
    fr = nc.dram_tensor("fr", [3, NF, NPIX], I32, kind="ExternalInput")
    hist = nc.dram_tensor("hist", [NF, 512], F32, kind="ExternalOutput")
    G = 2                # frames per op batch (amortizes per-op overhead)
    FD = G * FPP         # 784 free-dim elements per op

    with tile.TileContext(nc) as tc:
        with (
            tc.tile_pool(name="io", bufs=4) as io,
            tc.tile_pool(name="mid", bufs=2) as mid,
            tc.tile_pool(name="oh", bufs=2) as oh,
            tc.tile_pool(name="cst", bufs=1) as cst,
            tc.tile_pool(name="ps", bufs=4, space="PSUM") as ps,
        ):
            osb = cst.tile([V1, NF * V2], F32)  # [32, 800] result staging

            # per-ramp-column bias constants for the Act engine ([128,1] each)
            nbias = len(A_ACT) + len(B_ACT)
            bias_sb = cst.tile([P, max(nbias, 1)], F32)
            bias_ap = {}
            bi_i = 0
            for v in A_ACT:
                nc.gpsimd.memset(bias_sb[:, bi_i:bi_i + 1], -(v - 1) * 0.125)
                bias_ap[("A", v)] = bias_sb[:, bi_i:bi_i + 1]
                bi_i += 1
            for w in B_ACT:
                nc.gpsimd.memset(bias_sb[:, bi_i:bi_i + 1], -(w - 1) * 0.25)
                bias_ap[("B", w)] = bias_sb[:, bi_i:bi_i + 1]
                bi_i += 1


            for t0 in range(0, NF, G):
                r = io.tile([P, FD], I32, tag="ch")
                g = io.tile([P, FD], I32, tag="ch")
                b = io.tile([P, FD], I32, tag="ch")
                for ci, ch in ((0, r), (1, g), (2, b)):
                    nc.sync.dma_start(
                        out=ch[:].rearrange("p (q f) -> p q f", q=G),
                        in_=fr[ci, t0:t0 + G].rearrange("q (p f) -> p q f", p=P))

                # hi5 = ((R>>3)&28) | (G>>6) ; lo4 = ((G>>2)&8) | (B>>5)
                a2 = mid.tile([P, FD], I32, tag="t1")
                nc.vector.tensor_scalar(
                    out=a2[:], in0=r[:], scalar1=3, scalar2=28,
                    op0=OP.logical_shift_right, op1=OP.bitwise_and)
                hi_i = mid.tile([P, FD], I32, tag="hb")
                _stt_int(nc, hi_i[:], g[:], 6, a2[:],
                         OP.logical_shift_right, OP.bitwise_or)
                hi = mid.tile([P, FD], BF16, tag="hc")
                nc.scalar.copy(out=hi[:], in_=hi_i[:])
                c2 = mid.tile([P, FD], I32, tag="t2")
                nc.vector.tensor_scalar(
                    out=c2[:], in0=g[:], scalar1=2, scalar2=8,
                    op0=OP.logical_shift_right, op1=OP.bitwise_and)
                lo_i = mid.tile([P, FD], I32, tag="lb")
                _stt_int(nc, lo_i[:], b[:], 5, c2[:],
                         OP.logical_shift_right, OP.bitwise_or)

                A = oh.tile([P, V1 * FD], BF16, tag="A")
                for v in A_DVE:
                    nc.vector.tensor_scalar(
                        out=A[:, v * FD:(v + 1) * FD], in0=hi[:],
                        scalar1=float(v), scalar2=None, op0=OP.is_equal)
                for v in A_POOL:
                    nc.gpsimd.tensor_scalar(
                        out=A[:, v * FD:(v + 1) * FD], in0=hi_i[:],
                        scalar1=float(v), scalar2=None, op0=OP.is_equal)
                B = oh.tile([P, V2 * FD], BF16, tag="B")
                for w in B_ACT:
                    nc.scalar.activation(
                        out=B[:, w * FD:(w + 1) * FD], in_=lo_i[:],
                        func=ACT.Relu, bias=bias_ap[("B", w)], scale=0.25)
                for w in B_POOL:
                    nc.gpsimd.tensor_scalar(
                        out=B[:, w * FD:(w + 1) * FD], in0=lo_i[:],
                        scalar1=float(w), scalar2=None, op0=OP.is_equal)

                # contract over pixels per frame: M[u, w] += A_qj^T @ B_qj
                Aq = A[:].rearrange("p (v q f) -> p q f v", v=V1, q=G)
                Bq = B[:].rearrange("p (v q f) -> p q f v", v=V2, q=G)
                for q in range(G):
                    hps = ps.tile([V1, V2], F32)
                    for j in range(FPP):
                        nc.tensor.matmul(
                            out=hps[:],
                            lhsT=Aq[:, q, j, :],
                            rhs=Bq[:, q, j, :],
                            start=(j == 0), stop=(j == FPP - 1))
                    t = t0 + q
                    nc.vector.tensor_copy(
                        out=osb[:, t * V2:(t + 1) * V2], in_=hps[:])

            nc.sync.dma_start(
                out=hist[:].rearrange("t (u w) -> u t w", u=V1),
                in_=osb[:].rearrange("u (t w) -> u t w", w=V2))
    nc.compile()
    return nc


def build_fc_nc():
    """sim2 = xh @ xs^T [50,150]; win[t,l] = sim2[t, t+l]; out = relu(win@W^T + b)."""
    nc = bacc.Bacc("TRN2")
    # columns 0:50 = x_half^T, 50:200 = padded-context^T (one DMA -> one sem wait)
    xallT = nc.dram_tensor("xallT", [512, 200], BF16, kind="ExternalInput")
    wT = nc.dram_tensor("wT", [LW, P], F32, kind="ExternalInput")
    out = nc.dram_tensor("out", [NF, P], F32, kind="ExternalOutput")
    # rows written at stride 163 (sim2[t] at 163*t), diagonal read back at
    # stride 164: addr 164*t + l = 163*t + (t+l) = sim2[t, t+l]  (no overlap)
    scratch = nc.dram_tensor("scratch", [NF * 164], F32, kind="Internal")

    with tile.TileContext(nc) as tc:
        with (
            tc.tile_pool(name="sb", bufs=1) as sb,
            tc.tile_pool(name="ps", bufs=1, space="PSUM") as ps,
        ):
            xa_sb = sb.tile([P, 4 * 200], BF16)
            nc.sync.dma_start(
                out=xa_sb[:].rearrange("p (a t) -> p a t", a=4),
                in_=xallT[:].rearrange("(a p) t -> p a t", p=P))
            wt_sb = sb.tile([LW, P], F32)
            nc.sync.dma_start(out=wt_sb[:], in_=wT[:])

            sim_ps = ps.tile([NF, 150], F32)
            for a in range(4):
                nc.tensor.matmul(
                    out=sim_ps[:],
                    lhsT=xa_sb[:, a * 200:a * 200 + NF],
                    rhs=xa_sb[:, a * 200 + NF:(a + 1) * 200],
                    start=(a == 0), stop=(a == 3))
            sim_sb = sb.tile([NF, 150], F32)
            nc.vector.tensor_copy(out=sim_sb[:], in_=sim_ps[:])

            # row t of sim2 lands at flat offset 163*t
            nc.sync.dma_start(
                out=scratch[0:NF * 163].rearrange("(t c) -> t c", c=163)[:, 0:150],
                in_=sim_sb[:])
            # diagonal: win[t, l] = scratch[164*t + l] = sim2[t, t+l]
            win_sb = sb.tile([NF, LW], F32)
            nc.sync.dma_start(
                out=win_sb[:],
                in_=scratch[0:NF * 164].rearrange("(t c) -> t c", c=164)[:, 0:LW])

            # transpose win [50, 101] -> [101, 50] on the PE
            ident = sb.tile([NF, NF], F32)
            make_identity(nc, ident[:])
            win_ps = ps.tile([LW, NF], F32)
            nc.tensor.transpose(out=win_ps[:], in_=win_sb[:], identity=ident[:])
            win2 = sb.tile([LW, NF], F32)
            nc.vector.tensor_copy(out=win2[:], in_=win_ps[:])

            fc_ps = ps.tile([P, NF], F32)
            nc.tensor.matmul(out=fc_ps[:], lhsT=wt_sb[:], rhs=win2[:],
                             start=True, stop=True)
            res = sb.tile([P, NF], F32)
            nc.vector.tensor_copy(out=res[:], in_=fc_ps[:])
            # bias + relu applied on host (tiny); avoids a 2-wait Activation
            nc.sync.dma_start(out=out[:].rearrange("t o -> o t"), in_=res[:])
    nc.compile()
    return nc


_NC_CACHE = {}


def _get_nc(key, builder):
    if key not in _NC_CACHE:
        _NC_CACHE[key] = builder()
    return _NC_CACHE[key]


def kernel(frames, W, b):
    frames = np.asarray(frames, dtype=np.int32)
    W = np.asarray(W, dtype=np.float32)
    b = np.asarray(b, dtype=np.float32)
    Bn, _, T = frames.shape[:3]  # [4, 3, 100, 224, 224]

    nc_a = _get_nc("A", build_hist_nc)
    in_maps = []
    for c in range(NCORES):
        bi, h = c // 2, c % 2
        sl = frames[bi, :, h * NF:(h + 1) * NF].reshape(3, NF, NPIX)
        in_maps.append({"fr": np.ascontiguousarray(sl)})
    res_a = run_bass_kernel_spmd(nc_a, in_maps, list(range(NCORES))).results

    # exact correction of ramp-encoded columns: H = inv(VA^T) @ M @ inv(VB)
    VA, VB = encoding_mats()
    CA = np.linalg.inv(VA.T)
    CB = np.linalg.inv(VB)
    counts = np.zeros((Bn, T, 512), np.float32)
    for c in range(NCORES):
        bi, h = c // 2, c % 2
        M = res_a[c]["hist"].astype(np.float64).reshape(NF, V1, V2)
        H = np.einsum('uh,thl,lw->tuw', CA, M, CB)
        counts[bi, h * NF:(h + 1) * NF] = H.reshape(NF, 512)
    xn = counts / np.linalg.norm(counts, axis=2, keepdims=True)

    nc_b = _get_nc("B", build_fc_nc)
    wT = np.ascontiguousarray(W.T)           # [101, 128]
    in_maps = []
    for c in range(NCORES):
        bi, h = c // 2, c % 2
        t0 = h * NF
        xall = np.zeros((200, 512), np.float32)
        xall[0:NF] = xn[bi, t0:t0 + NF]                  # x_half
        xall[NF + 50 - t0:NF + 50 - t0 + T] = xn[bi]     # xs[s'] = xn[s'+t0-50]
        import ml_dtypes
        xT = np.ascontiguousarray(xall.T).astype(ml_dtypes.bfloat16)
        in_maps.append({"xallT": xT, "wT": wT})
    res_b = run_bass_kernel_spmd(nc_b, in_maps, list(range(NCORES))).results

    outp = np.zeros((Bn, T, P), np.float32)
    for c in range(NCORES):
        bi, h = c // 2, c % 2
        outp[bi, h * NF:(h + 1) * NF] = res_b[c]["out"]
    outp = np.maximum(outp + b[None, None, :], 0.0)
    return outp


# revision 31
# speedup vs baseline: 1.5463x; 1.0063x over previous
"""Trainium2 Bass kernel for nn_AutoShot (histogram binning + windowed similarity + FC).

Sharding: data-parallel over B*T = 400 frames -> 8 cores x 50 frames.
Phase A (heavy): per-core color histograms [50, 512] via
  bin = (R>>5)<<6 | (G>>5)<<3 | (B>>5), split bin = hi5*16 + lo4,
  encoding matrices A [px,32], B [px,16] contracted over pixels on the
  PE (PSUM-accumulated bf16 matmuls) -> joint 2-D histogram [32,16].
  Encoding columns are split across three engines to run concurrently:
    - DVE:  is_equal one-hot columns (4x perf mode, bf16)
    - Act:  relu-ramp columns relu((x-s)*c) - exact linear correction on host
    - Pool: is_equal one-hot columns
  The per-frame [32,16] raw moment matrix M = A^T B is corrected on the host:
  H = inv(VA^T) @ M @ inv(VB) where VA/VB are the exact encoding matrices
  (integer/eighth-integer entries, exact in bf16/fp32; correction in f64).
Phase B (light): per-core sim = xh @ xs^T (xs = zero-padded +-50 frame context),
  diagonal window extract via a stride-164 read over stride-163 rows in a DRAM
  scratch (addr 164*t + l = sim[t, t+l]), PE transpose, FC matmul (W [128,101]).
Host: slices inputs, applies correction, L2-normalizes histograms between
  launches, applies bias + ReLU (tiny [400,128] tail), reassembles output.
"""

import sys

for _p in ("/opt/trn_rl_repo", "/root/.axon_site/_ro/trn_rl_repo"):
    if _p not in sys.path:
        sys.path.append(_p)

import numpy as np

from concourse import bass, bacc, mybir
import concourse.tile as tile
from concourse.bass_utils import run_bass_kernel_spmd
from concourse.masks import make_identity

P = 128
NPIX = 224 * 224        # 50176 pixels per frame plane
FPP = NPIX // P         # 392 pixels per partition
NF = 50                 # frames per core
V1, V2 = 32, 16         # 512 = 32 * 16 bin split
LW = 101
NCORES = 8
F32 = mybir.dt.float32
I32 = mybir.dt.int32
I16 = mybir.dt.int16
BF16 = mybir.dt.bfloat16
OP = mybir.AluOpType
ACT = mybir.ActivationFunctionType

# Column assignment across engines (balanced by cost model rates):
#  DVE is_equal col: 265ns/batch; Act ramp col: 838ns; Pool is_equal col: 1090ns
A_DVE = list(range(28))       # A-side one-hot columns on DVE
A_POOL = list(range(28, 32))  # A-side one-hot columns on Pool (imm scalar)
A_ACT = []                    # (unused) A-side ramp cols
B_DVE = [11]                  # B-side one-hot column on DVE (bf16 lo)
B_ACT = list(range(11))       # B-side ramp cols: relu(l-(w-1)), w=0..10
B_SPLIT = []                  # (unused) fractional split columns
B_POOL = list(range(12, 16))  # B-side one-hot columns on Pool (imm scalar)


def encoding_mats():
    """Exact encoding matrices VA [32,32], VB [16,16]: col c of VA evaluated
    at value h is the device-computed encoding A[pix,c] for hi==h."""
    h = np.arange(32, dtype=np.float64)
    VA = np.zeros((32, 32))
    for c in A_DVE:
        VA[c, c] = 1.0
    for c in A_POOL:
        VA[c, c] = 1.0
    for c in A_ACT:
        VA[:, c] = np.maximum((h - (c - 1)) * 0.125, 0.0)
    ll = np.arange(16, dtype=np.float64)
    VB = np.zeros((16, 16))
    for c in B_ACT + B_SPLIT:
        VB[:, c] = np.maximum(ll - (c - 1), 0.0)
    for c in B_DVE + B_POOL:
        VB[c, c] = 1.0
    return VA, VB


def _stt_int(nc, out, in0, scalar_int, in1, op0, op1):
    """scalar_tensor_tensor with an int32 immediate: out = (in0 op0 s) op1 in1."""
    v = nc.vector
    return v.add_instruction(mybir.InstTensorScalarPtr(
        name=v.bass.get_next_instruction_name(),
        is_scalar_tensor_tensor=True,
        op0=op0, op1=op1,
        ins=[v.lower_ap(in0),
             mybir.ImmediateValue(dtype=mybir.dt.int32, value=scalar_int),
             v.lower_ap(in1)],
        outs=[v.lower_ap(out)],
    ))


def build_hist_nc():
    nc = bacc.Bacc("TRN2")
    fr = nc.dram_tensor("fr", [3, NF, NPIX], I32, kind="ExternalInput")
    hist = nc.dram_tensor("hist", [NF, 512], F32, kind="ExternalOutput")
    G = 2                # frames per op batch (amortizes per-op overhead)
    FD = G * FPP         # 784 free-dim elements per op

    with tile.TileContext(nc) as tc:
        with (
            tc.tile_pool(name="io", bufs=4) as io,
            tc.tile_pool(name="mid", bufs=2) as mid,
            tc.tile_pool(name="oh", bufs=2) as oh,
            tc.tile_pool(name="cst", bufs=1) as cst,
            tc.tile_pool(name="ps", bufs=4, space="PSUM") as ps,
        ):
            osb = cst.tile([V1, NF * V2], F32)  # [32, 800] result staging

            # per-ramp-column bias constants for the Act engine ([128,1] each)
            nbias = len(A_ACT) + len(B_ACT) + len(B_SPLIT)
            bias_sb = cst.tile([P, max(nbias, 1)], F32)
            bias_ap = {}
            bi_i = 0
            for v in A_ACT:
                nc.gpsimd.memset(bias_sb[:, bi_i:bi_i + 1], -(v - 1) * 0.125)
                bias_ap[("A", v)] = bias_sb[:, bi_i:bi_i + 1]
                bi_i += 1
            for w in B_ACT + B_SPLIT:
                nc.gpsimd.memset(bias_sb[:, bi_i:bi_i + 1], -float(w - 1))
                bias_ap[("B", w)] = bias_sb[:, bi_i:bi_i + 1]
                bi_i += 1


            # variable-size frame groups: small first/last batches shrink
            # pipeline fill and PE drain
            groups = [1, 1] + [2] * ((NF - 4) // 2) + [1, 1]
            t0 = 0
            out_done = 0
            def emit_cols_and_matmuls(st):
                (Gc, FDc, t0, hi, hi_i, lo, lo_i, A, B) = st
                for v in A_DVE:
                    nc.vector.tensor_scalar(
                        out=A[:, v * FDc:(v + 1) * FDc], in0=hi[:],
                        scalar1=float(v), scalar2=None, op0=OP.is_equal)
                for v in A_POOL:
                    nc.gpsimd.tensor_scalar(
                        out=A[:, v * FDc:(v + 1) * FDc], in0=hi_i[:],
                        scalar1=float(v), scalar2=None, op0=OP.is_equal)
                for w in B_ACT:
                    nc.scalar.activation(
                        out=B[:, w * FDc:(w + 1) * FDc], in_=lo_i[:],
                        func=ACT.Relu, bias=bias_ap[("B", w)], scale=1.0)
                for w in B_DVE:
                    nc.vector.tensor_scalar(
                        out=B[:, w * FDc:(w + 1) * FDc], in0=lo[:],
                        scalar1=float(w), scalar2=None, op0=OP.is_equal)
                for w in B_POOL:
                    nc.gpsimd.tensor_scalar(
                        out=B[:, w * FDc:(w + 1) * FDc], in0=lo_i[:],
                        scalar1=float(w), scalar2=None, op0=OP.is_equal)
                Aq = A[:].rearrange("p (v q f) -> p q f v", v=V1, q=Gc)
                Bq = B[:].rearrange("p (v q f) -> p q f v", v=V2, q=Gc)
                for q in range(Gc):
                    hps = ps.tile([V1, V2], F32)
                    for j in range(FPP):
                        nc.tensor.matmul(
                            out=hps[:],
                            lhsT=Aq[:, q, j, :],
                            rhs=Bq[:, q, j, :],
                            start=(j == 0), stop=(j == FPP - 1))
                    t = t0 + q
                    nc.vector.tensor_copy(
                        out=osb[:, t * V2:(t + 1) * V2], in_=hps[:])
                return t0 + Gc

            pend = None
            for gi, Gc in enumerate(groups):
                FDc = Gc * FPP
                r = io.tile([P, FDc], I32, tag="ch")
                g = io.tile([P, FDc], I32, tag="ch")
                b = io.tile([P, FDc], I32, tag="ch")
                for ci, ch in ((0, r), (1, g), (2, b)):
                    nc.sync.dma_start(
                        out=ch[:].rearrange("p (q f) -> p q f", q=Gc),
                        in_=fr[ci, t0:t0 + Gc].rearrange("q (p f) -> p q f", p=P))

                # hi5 = ((R>>3)&28) | (G>>6) ; lo4 = ((G>>2)&8) | (B>>5)
                a2 = mid.tile([P, FDc], I32, tag="t1")
                nc.vector.tensor_scalar(
                    out=a2[:], in0=r[:], scalar1=3, scalar2=28,
                    op0=OP.logical_shift_right, op1=OP.bitwise_and)
                hi_i = mid.tile([P, FDc], I32, tag="hb")
                _stt_int(nc, hi_i[:], g[:], 6, a2[:],
                         OP.logical_shift_right, OP.bitwise_or)
                hi = mid.tile([P, FDc], BF16, tag="hc")
                nc.scalar.copy(out=hi[:], in_=hi_i[:])
                c2 = mid.tile([P, FDc], I32, tag="t2")
                nc.vector.tensor_scalar(
                    out=c2[:], in0=g[:], scalar1=2, scalar2=8,
                    op0=OP.logical_shift_right, op1=OP.bitwise_and)
                lo_i = mid.tile([P, FDc], I32, tag="lb")
                _stt_int(nc, lo_i[:], b[:], 5, c2[:],
                         OP.logical_shift_right, OP.bitwise_or)
                lo = mid.tile([P, FDc], BF16, tag="lc")
                nc.scalar.copy(out=lo[:], in_=lo_i[:])

                A = oh.tile([P, V1 * FDc], BF16, tag="A")
                B = oh.tile([P, V2 * FDc], BF16, tag="B")
                st = (Gc, FDc, t0, hi, hi_i, lo, lo_i, A, B)
                t0 += Gc
                if pend is not None:
                    done = emit_cols_and_matmuls(pend)
                    while out_done + 10 <= done:
                        oc = 10
                        nc.sync.dma_start(
                            out=hist[out_done:out_done + oc].rearrange(
                                "t (u w) -> u t w", u=V1),
                            in_=osb[:, out_done * V2:(out_done + oc) * V2].rearrange(
                                "u (t w) -> u t w", w=V2))
                        out_done += oc
                pend = st
            done = emit_cols_and_matmuls(pend)
            while out_done < NF:
                oc = min(10, NF - out_done)
                nc.sync.dma_start(
                    out=hist[out_done:out_done + oc].rearrange(
                        "t (u w) -> u t w", u=V1),
                    in_=osb[:, out_done * V2:(out_done + oc) * V2].rearrange(
                        "u (t w) -> u t w", w=V2))
                out_done += oc
    nc.compile()
    return nc


def build_fc_nc():
    """sim2 = xh @ xs^T [50,150]; win[t,l] = sim2[t, t+l]; out = relu(win@W^T + b)."""
    nc = bacc.Bacc("TRN2")
    # columns 0:50 = x_half^T, 50:200 = padded-context^T (one DMA -> one sem wait)
    xallT = nc.dram_tensor("xallT", [512, 200], BF16, kind="ExternalInput")
    wT = nc.dram_tensor("wT", [LW, P], F32, kind="ExternalInput")
    out = nc.dram_tensor("out", [NF, P], F32, kind="ExternalOutput")
    # rows written at stride 163 (sim2[t] at 163*t), diagonal read back at
    # stride 164: addr 164*t + l = 163*t + (t+l) = sim2[t, t+l]  (no overlap)
    scratch = nc.dram_tensor("scratch", [NF * 164], F32, kind="Internal")

    with tile.TileContext(nc) as tc:
        with (
            tc.tile_pool(name="sb", bufs=1) as sb,
            tc.tile_pool(name="ps", bufs=1, space="PSUM") as ps,
        ):
            xa_sb = sb.tile([P, 4 * 200], BF16)
            nc.sync.dma_start(
                out=xa_sb[:].rearrange("p (a t) -> p a t", a=4),
                in_=xallT[:].rearrange("(a p) t -> p a t", p=P))
            wt_sb = sb.tile([LW, P], F32)
            nc.sync.dma_start(out=wt_sb[:], in_=wT[:])

            sim_ps = ps.tile([NF, 150], F32)
            for a in range(4):
                nc.tensor.matmul(
                    out=sim_ps[:],
                    lhsT=xa_sb[:, a * 200:a * 200 + NF],
                    rhs=xa_sb[:, a * 200 + NF:(a + 1) * 200],
                    start=(a == 0), stop=(a == 3))
            sim_sb = sb.tile([NF, 150], F32)
            nc.vector.tensor_copy(out=sim_sb[:], in_=sim_ps[:])

            # row t of sim2 lands at flat offset 163*t
            nc.sync.dma_start(
                out=scratch[0:NF * 163].rearrange("(t c) -> t c", c=163)[:, 0:150],
                in_=sim_sb[:])
            # diagonal: win[t, l] = scratch[164*t + l] = sim2[t, t+l]
            win_sb = sb.tile([NF, LW], F32)
            nc.sync.dma_start(
                out=win_sb[:],
                in_=scratch[0:NF * 164].rearrange("(t c) -> t c", c=164)[:, 0:LW])

            # transpose win [50, 101] -> [101, 50] on the PE
            ident = sb.tile([NF, NF], F32)
            make_identity(nc, ident[:])
            win_ps = ps.tile([LW, NF], F32)
            nc.tensor.transpose(out=win_ps[:], in_=win_sb[:], identity=ident[:])
            win2 = sb.tile([LW, NF], F32)
            nc.vector.tensor_copy(out=win2[:], in_=win_ps[:])

            fc_ps = ps.tile([P, NF], F32)
            nc.tensor.matmul(out=fc_ps[:], lhsT=wt_sb[:], rhs=win2[:],
                             start=True, stop=True)
            res = sb.tile([P, NF], F32)
            nc.vector.tensor_copy(out=res[:], in_=fc_ps[:])
            # bias + relu applied on host (tiny); avoids a 2-wait Activation
            nc.sync.dma_start(out=out[:].rearrange("t o -> o t"), in_=res[:])
    nc.compile()
    return nc


_NC_CACHE = {}


def _get_nc(key, builder):
    if key not in _NC_CACHE:
        _NC_CACHE[key] = builder()
    return _NC_CACHE[key]


def kernel(frames, W, b):
    frames = np.asarray(frames, dtype=np.int32)
    W = np.asarray(W, dtype=np.float32)
    b = np.asarray(b, dtype=np.float32)
    Bn, _, T = frames.shape[:3]  # [4, 3, 100, 224, 224]

    nc_a = _get_nc("A", build_hist_nc)
    in_maps = []
    for c in range(NCORES):
        bi, h = c // 2, c % 2
        sl = frames[bi, :, h * NF:(h + 1) * NF].reshape(3, NF, NPIX)
        in_maps.append({"fr": np.ascontiguousarray(sl)})
    res_a = run_bass_kernel_spmd(nc_a, in_maps, list(range(NCORES))).results

    # exact correction of ramp-encoded columns: H = inv(VA^T) @ M @ inv(VB)
    VA, VB = encoding_mats()
    CA = np.linalg.inv(VA.T)
    CB = np.linalg.inv(VB)
    counts = np.zeros((Bn, T, 512), np.float32)
    for c in range(NCORES):
        bi, h = c // 2, c % 2
        M = res_a[c]["hist"].astype(np.float64).reshape(NF, V1, V2)
        H = np.einsum('uh,thl,lw->tuw', CA, M, CB)
        counts[bi, h * NF:(h + 1) * NF] = H.reshape(NF, 512)
    xn = counts / np.linalg.norm(counts, axis=2, keepdims=True)

    nc_b = _get_nc("B", build_fc_nc)
    wT = np.ascontiguousarray(W.T)           # [101, 128]
    in_maps = []
    for c in range(NCORES):
        bi, h = c // 2, c % 2
        t0 = h * NF
        xall = np.zeros((200, 512), np.float32)
        xall[0:NF] = xn[bi, t0:t0 + NF]                  # x_half
        xall[NF + 50 - t0:NF + 50 - t0 + T] = xn[bi]     # xs[s'] = xn[s'+t0-50]
        import ml_dtypes
        xT = np.ascontiguousarray(xall.T).astype(ml_dtypes.bfloat16)
        in_maps.append({"xallT": xT, "wT": wT})
    res_b = run_bass_kernel_spmd(nc_b, in_maps, list(range(NCORES))).results

    outp = np.zeros((Bn, T, P), np.float32)
    for c in range(NCORES):
        bi, h = c // 2, c % 2
        outp[bi, h * NF:(h + 1) * NF] = res_b[c]["out"]
    outp = np.maximum(outp + b[None, None, :], 0.0)
    return outp


# revision 37
# speedup vs baseline: 1.6159x; 1.0450x over previous
"""Trainium2 Bass kernel for nn_AutoShot (histogram binning + windowed similarity + FC).

Sharding: data-parallel over B*T = 400 frames -> 8 cores x 50 frames.
Phase A (heavy): per-core color histograms [50, 512] via
  bin = (R>>5)<<6 | (G>>5)<<3 | (B>>5), split bin = hi5*16 + lo4,
  encoding matrices A [px,32], B [px,16] contracted over pixels on the
  PE (PSUM-accumulated bf16 matmuls) -> joint 2-D histogram [32,16].
  Encoding columns are split across three engines to run concurrently:
    - DVE:  is_equal one-hot columns (4x perf mode, bf16)
    - Act:  relu-ramp columns relu((x-s)*c) - exact linear correction on host
    - Pool: is_equal one-hot columns
  The per-frame [32,16] raw moment matrix M = A^T B is corrected on the host:
  H = inv(VA^T) @ M @ inv(VB) where VA/VB are the exact encoding matrices
  (integer/eighth-integer entries, exact in bf16/fp32; correction in f64).
Phase B (light): per-core sim = xh @ xs^T (xs = zero-padded +-50 frame context),
  diagonal window extract via a stride-164 read over stride-163 rows in a DRAM
  scratch (addr 164*t + l = sim[t, t+l]), PE transpose, FC matmul (W [128,101]).
Host: slices inputs, applies correction, L2-normalizes histograms between
  launches, applies bias + ReLU (tiny [400,128] tail), reassembles output.
"""

import sys

for _p in ("/opt/trn_rl_repo", "/root/.axon_site/_ro/trn_rl_repo"):
    if _p not in sys.path:
        sys.path.append(_p)

import numpy as np

from concourse import bass, bacc, mybir
import concourse.tile as tile
from concourse.bass_utils import run_bass_kernel_spmd
from concourse.masks import make_identity

P = 128
NPIX = 224 * 224        # 50176 pixels per frame plane
FPP = NPIX // P         # 392 pixels per partition
NF = 50                 # frames per core
V1, V2 = 32, 16         # 512 = 32 * 16 bin split
LW = 101
NCORES = 8
F32 = mybir.dt.float32
I32 = mybir.dt.int32
I16 = mybir.dt.int16
BF16 = mybir.dt.bfloat16
OP = mybir.AluOpType
ACT = mybir.ActivationFunctionType

# Column assignment across engines (balanced by cost model rates):
#  DVE is_equal col: 265ns/batch; Act ramp col: 838ns; Pool is_equal col: 1090ns
A_DVE = list(range(28))       # A-side one-hot columns on DVE
A_POOL = list(range(28, 32))  # A-side one-hot columns on Pool (imm scalar)
A_ACT = []                    # (unused) A-side ramp cols
B_DVE = [11]                  # B-side one-hot column on DVE (int32 lo)
B_ACT = list(range(11))       # B-side ramp cols: relu(l-(w-1)), w=0..10
B_SPLIT = []                  # (unused) fractional split columns
B_POOL = list(range(12, 16))  # B-side one-hot columns on Pool (imm scalar)


def encoding_mats():
    """Exact encoding matrices VA [32,32], VB [16,16]: col c of VA evaluated
    at value h is the device-computed encoding A[pix,c] for hi==h."""
    h = np.arange(32, dtype=np.float64)
    VA = np.zeros((32, 32))
    for c in A_DVE:
        VA[c, c] = 1.0
    for c in A_POOL:
        VA[c, c] = 1.0
    for c in A_ACT:
        VA[:, c] = np.maximum((h - (c - 1)) * 0.125, 0.0)
    ll = np.arange(16, dtype=np.float64)
    VB = np.zeros((16, 16))
    for c in B_ACT + B_SPLIT:
        VB[:, c] = np.maximum(ll - (c - 1), 0.0)
    for c in B_DVE + B_POOL:
        VB[c, c] = 1.0
    return VA, VB


def _stt_int(nc, out, in0, scalar_int, in1, op0, op1):
    """scalar_tensor_tensor with an int32 immediate: out = (in0 op0 s) op1 in1."""
    v = nc.vector
    return v.add_instruction(mybir.InstTensorScalarPtr(
        name=v.bass.get_next_instruction_name(),
        is_scalar_tensor_tensor=True,
        op0=op0, op1=op1,
        ins=[v.lower_ap(in0),
             mybir.ImmediateValue(dtype=mybir.dt.int32, value=scalar_int),
             v.lower_ap(in1)],
        outs=[v.lower_ap(out)],
    ))


def build_hist_nc():
    nc = bacc.Bacc("TRN2")
    fr = nc.dram_tensor("fr", [3, NF, NPIX], I32, kind="ExternalInput")
    hist = nc.dram_tensor("hist", [NF, 512], F32, kind="ExternalOutput")
    G = 2                # frames per op batch (amortizes per-op overhead)
    FD = G * FPP         # 784 free-dim elements per op

    with tile.TileContext(nc) as tc:
        with (
            tc.tile_pool(name="io", bufs=4) as io,
            tc.tile_pool(name="mid", bufs=2) as mid,
            tc.tile_pool(name="oh", bufs=2) as oh,
            tc.tile_pool(name="cst", bufs=1) as cst,
            tc.tile_pool(name="ps", bufs=4, space="PSUM") as ps,
        ):
            osb = cst.tile([V1, NF * V2], F32)  # [32, 800] result staging

            # per-ramp-column bias constants for the Act engine ([128,1] each)
            nbias = len(A_ACT) + len(B_ACT) + len(B_SPLIT)
            bias_sb = cst.tile([P, max(nbias, 1)], F32)
            bias_ap = {}
            bi_i = 0
            for v in A_ACT:
                nc.gpsimd.memset(bias_sb[:, bi_i:bi_i + 1], -(v - 1) * 0.125)
                bias_ap[("A", v)] = bias_sb[:, bi_i:bi_i + 1]
                bi_i += 1
            for w in B_ACT + B_SPLIT:
                nc.gpsimd.memset(bias_sb[:, bi_i:bi_i + 1], -float(w - 1))
                bias_ap[("B", w)] = bias_sb[:, bi_i:bi_i + 1]
                bi_i += 1


            # variable-size frame groups: small first/last batches shrink
            # pipeline fill and PE drain
            groups = [1, 1] + [2] * ((NF - 4) // 2) + [1, 1]
            t0 = 0
            out_done = 0
            def emit_cols_and_matmuls(st):
                (Gc, FDc, t0, hi, hi_i, lo, lo_i, A, B) = st
                for v in A_DVE:
                    nc.vector.tensor_scalar(
                        out=A[:, v * FDc:(v + 1) * FDc], in0=hi[:],
                        scalar1=float(v), scalar2=None, op0=OP.is_equal)
                for v in A_POOL:
                    nc.gpsimd.tensor_scalar(
                        out=A[:, v * FDc:(v + 1) * FDc], in0=hi_i[:],
                        scalar1=float(v), scalar2=None, op0=OP.is_equal)
                for w in B_ACT:
                    nc.scalar.activation(
                        out=B[:, w * FDc:(w + 1) * FDc], in_=lo_i[:],
                        func=ACT.Relu, bias=bias_ap[("B", w)], scale=1.0)
                for w in B_DVE:
                    nc.vector.tensor_scalar(
                        out=B[:, w * FDc:(w + 1) * FDc], in0=lo[:],
                        scalar1=float(w), scalar2=None, op0=OP.is_equal)
                for w in B_POOL:
                    nc.gpsimd.tensor_scalar(
                        out=B[:, w * FDc:(w + 1) * FDc], in0=lo_i[:],
                        scalar1=float(w), scalar2=None, op0=OP.is_equal)
                Aq = A[:].rearrange("p (v q f) -> p q f v", v=V1, q=Gc)
                Bq = B[:].rearrange("p (v q f) -> p q f v", v=V2, q=Gc)
                for q in range(Gc):
                    hps = ps.tile([V1, V2], F32)
                    for j in range(FPP):
                        nc.tensor.matmul(
                            out=hps[:],
                            lhsT=Aq[:, q, j, :],
                            rhs=Bq[:, q, j, :],
                            start=(j == 0), stop=(j == FPP - 1))
                    t = t0 + q
                    nc.scalar.copy(
                        out=osb[:, t * V2:(t + 1) * V2], in_=hps[:])
                return t0 + Gc

            pend = None
            for gi, Gc in enumerate(groups):
                FDc = Gc * FPP
                r = io.tile([P, FDc], I32, tag="ch")
                g = io.tile([P, FDc], I32, tag="ch")
                b = io.tile([P, FDc], I32, tag="ch")
                for ci, ch in ((0, r), (1, g), (2, b)):
                    nc.sync.dma_start(
                        out=ch[:].rearrange("p (q f) -> p q f", q=Gc),
                        in_=fr[ci, t0:t0 + Gc].rearrange("q (p f) -> p q f", p=P))

                # hi5 = ((R>>3)&28) | (G>>6) ; lo4 = ((G>>2)&8) | (B>>5)
                a2 = mid.tile([P, FDc], I32, tag="t1")
                nc.vector.tensor_scalar(
                    out=a2[:], in0=r[:], scalar1=3, scalar2=28,
                    op0=OP.logical_shift_right, op1=OP.bitwise_and)
                hi_i = mid.tile([P, FDc], I32, tag="hb")
                _stt_int(nc, hi_i[:], g[:], 6, a2[:],
                         OP.logical_shift_right, OP.bitwise_or)
                hi = mid.tile([P, FDc], BF16, tag="hc")
                nc.scalar.copy(out=hi[:], in_=hi_i[:])
                c2 = mid.tile([P, FDc], I32, tag="t2")
                nc.vector.tensor_scalar(
                    out=c2[:], in0=g[:], scalar1=2, scalar2=8,
                    op0=OP.logical_shift_right, op1=OP.bitwise_and)
                lo_i = mid.tile([P, FDc], I32, tag="lb")
                _stt_int(nc, lo_i[:], b[:], 5, c2[:],
                         OP.logical_shift_right, OP.bitwise_or)
                lo = lo_i  # B col reads int32 directly (2x mode, no convert)

                A = oh.tile([P, V1 * FDc], BF16, tag="A")
                B = oh.tile([P, V2 * FDc], BF16, tag="B")
                st = (Gc, FDc, t0, hi, hi_i, lo, lo_i, A, B)
                t0 += Gc
                if pend is not None:
                    done = emit_cols_and_matmuls(pend)
                    while out_done + 10 <= done:
                        oc = 10
                        nc.sync.dma_start(
                            out=hist[out_done:out_done + oc].rearrange(
                                "t (u w) -> u t w", u=V1),
                            in_=osb[:, out_done * V2:(out_done + oc) * V2].rearrange(
                                "u (t w) -> u t w", w=V2))
                        out_done += oc
                pend = st
            done = emit_cols_and_matmuls(pend)
            while out_done < NF:
                oc = min(10, NF - out_done)
                nc.sync.dma_start(
                    out=hist[out_done:out_done + oc].rearrange(
                        "t (u w) -> u t w", u=V1),
                    in_=osb[:, out_done * V2:(out_done + oc) * V2].rearrange(
                        "u (t w) -> u t w", w=V2))
                out_done += oc
    nc.compile()
    return nc


def build_fc_nc():
    """sim2 = xh @ xs^T [50,150]; win[t,l] = sim2[t, t+l]; out = relu(win@W^T + b)."""
    nc = bacc.Bacc("TRN2")
    # columns 0:50 = x_half^T, 50:200 = padded-context^T (one DMA -> one sem wait)
    xallT = nc.dram_tensor("xallT", [512, 200], BF16, kind="ExternalInput")
    wT = nc.dram_tensor("wT", [LW, P], F32, kind="ExternalInput")
    out = nc.dram_tensor("out", [P, NF], F32, kind="ExternalOutput")
    # rows written at stride 163 (sim2[t] at 163*t), diagonal read back at
    # stride 164: addr 164*t + l = 163*t + (t+l) = sim2[t, t+l]  (no overlap)
    scratch = nc.dram_tensor("scratch", [NF * 164], F32, kind="Internal")

    with tile.TileContext(nc) as tc:
        with (
            tc.tile_pool(name="sb", bufs=1) as sb,
            tc.tile_pool(name="ps", bufs=1, space="PSUM") as ps,
        ):
            xa_sb = sb.tile([P, 4 * 200], BF16)
            nc.sync.dma_start(
                out=xa_sb[:].rearrange("p (a t) -> p a t", a=4),
                in_=xallT[:].rearrange("(a p) t -> p a t", p=P))
            wt_sb = sb.tile([LW, P], F32)
            nc.sync.dma_start(out=wt_sb[:], in_=wT[:])

            sim_ps = ps.tile([NF, 150], F32)
            for a in range(4):
                nc.tensor.matmul(
                    out=sim_ps[:],
                    lhsT=xa_sb[:, a * 200:a * 200 + NF],
                    rhs=xa_sb[:, a * 200 + NF:(a + 1) * 200],
                    start=(a == 0), stop=(a == 3))
            sim_sb = sb.tile([NF, 150], F32)
            nc.vector.tensor_copy(out=sim_sb[:], in_=sim_ps[:])

            # row t of sim2 lands at flat offset 163*t
            nc.sync.dma_start(
                out=scratch[0:NF * 163].rearrange("(t c) -> t c", c=163)[:, 0:150],
                in_=sim_sb[:])
            # diagonal: win[t, l] = scratch[164*t + l] = sim2[t, t+l]
            win_sb = sb.tile([NF, LW], F32)
            nc.sync.dma_start(
                out=win_sb[:],
                in_=scratch[0:NF * 164].rearrange("(t c) -> t c", c=164)[:, 0:LW])

            # transpose win [50, 101] -> [101, 50] on the PE
            ident = sb.tile([NF, NF], F32)
            make_identity(nc, ident[:])
            win_ps = ps.tile([LW, NF], F32)
            nc.tensor.transpose(out=win_ps[:], in_=win_sb[:], identity=ident[:])
            win2 = sb.tile([LW, NF], F32)
            nc.vector.tensor_copy(out=win2[:], in_=win_ps[:])

            fc_ps = ps.tile([P, NF], F32)
            nc.tensor.matmul(out=fc_ps[:], lhsT=wt_sb[:], rhs=win2[:],
                             start=True, stop=True)
            res = sb.tile([P, NF], F32)
            nc.vector.tensor_copy(out=res[:], in_=fc_ps[:])
            # bias + relu + transpose applied on host (tiny)
            nc.sync.dma_start(out=out[:], in_=res[:])
    nc.compile()
    return nc


_NC_CACHE = {}


def _get_nc(key, builder):
    if key not in _NC_CACHE:
        _NC_CACHE[key] = builder()
    return _NC_CACHE[key]


def kernel(frames, W, b):
    frames = np.asarray(frames, dtype=np.int32)
    W = np.asarray(W, dtype=np.float32)
    b = np.asarray(b, dtype=np.float32)
    Bn, _, T = frames.shape[:3]  # [4, 3, 100, 224, 224]

    nc_a = _get_nc("A", build_hist_nc)
    in_maps = []
    for c in range(NCORES):
        bi, h = c // 2, c % 2
        sl = frames[bi, :, h * NF:(h + 1) * NF].reshape(3, NF, NPIX)
        in_maps.append({"fr": np.ascontiguousarray(sl)})
    res_a = run_bass_kernel_spmd(nc_a, in_maps, list(range(NCORES))).results

    # exact correction of ramp-encoded columns: H = inv(VA^T) @ M @ inv(VB)
    VA, VB = encoding_mats()
    CA = np.linalg.inv(VA.T)
    CB = np.linalg.inv(VB)
    counts = np.zeros((Bn, T, 512), np.float32)
    for c in range(NCORES):
        bi, h = c // 2, c % 2
        M = res_a[c]["hist"].astype(np.float64).reshape(NF, V1, V2)
        H = np.einsum('uh,thl,lw->tuw', CA, M, CB)
        counts[bi, h * NF:(h + 1) * NF] = H.reshape(NF, 512)
    xn = counts / np.linalg.norm(counts, axis=2, keepdims=True)

    nc_b = _get_nc("B", build_fc_nc)
    wT = np.ascontiguousarray(W.T)           # [101, 128]
    in_maps = []
    for c in range(NCORES):
        bi, h = c // 2, c % 2
        t0 = h * NF
        xall = np.zeros((200, 512), np.float32)
        xall[0:NF] = xn[bi, t0:t0 + NF]                  # x_half
        xall[NF + 50 - t0:NF + 50 - t0 + T] = xn[bi]     # xs[s'] = xn[s'+t0-50]
        import ml_dtypes
        xT = np.ascontiguousarray(xall.T).astype(ml_dtypes.bfloat16)
        in_maps.append({"xallT": xT, "wT": wT})
    res_b = run_bass_kernel_spmd(nc_b, in_maps, list(range(NCORES))).results

    outp = np.zeros((Bn, T, P), np.float32)
    for c in range(NCORES):
        bi, h = c // 2, c % 2
        outp[bi, h * NF:(h + 1) * NF] = res_b[c]["out"].T
    outp = np.maximum(outp + b[None, None, :], 0.0)
    return outp
